# revision 5
# baseline (speedup 1.0000x reference)
"""ACE layer (moe_routing) Trainium2 kernel — 8 NeuronCores, data-parallel over atoms.

out[i] = sum_{c,a,b,m} w_e[c,a,b] * ct_e[a,b,m] * f0[i,c,a,m] * f1[i,c,b,m],  e = element_ids[i]

Strategy:
  host: A_e[c,a,b,m] = w_e * ct_e (tiny). Sort atoms by element; every core gets an
        equal contiguous chunk of EVERY element's atoms (uniform SPMD program,
        perfectly balanced). Pack f0/f1 (bf16) into [128 x cols] tiles whose
        partition layout matches the PE 32-row block structure per element:
          el0/el2 (n=32,lm=25): blocks = 4 m's per tile, 6 tiles/c + m=24 packed 4-c tiles
          el1     (n=24,lm=16): blocks = 4 m's per tile (rows 24:32 zero), 4 tiles/c
          el3     (n=16,lm=9) : blocks = 2 m's stacked per 32 rows, 1 tile/c + m=8 packed
  device, per tile pair: 8 diagonal-tile_position 32x32 matmuls G = A^T f1 (PSUM),
        one DVE multiply h = f0 * G (bf16), 2 ones-matmuls (K=128,M=1) accumulating
        sum_p h[p,i] into 4 column-group slots of a per-element PSUM accumulator.
  host: gather per-core outputs, sum the 4 column-group rows, scatter to atom order.
"""

import math
import os

import numpy as np
import ml_dtypes

BF16 = ml_dtypes.bfloat16
SPECS = [(32, 25), (24, 16), (32, 25), (16, 9)]
N_ATOMS, C, NMAX, LMMAX, E = 8192, 32, 32, 25, 4
NCORES = 8


# ---------------------------------------------------------------- schedule --
def _schedule(caps):
    """Per element: list of groups. Each group is one SBUF tile (one DMA per
    f-tensor) containing `nu` units; unit = one [128 x cap] column block with
    its own lhsT [128 x 32] tile. Returns (segments, total_units).

    group: dict(key, idx, nu)  -> dram param f'{t}_{key}' indexed [idx] (or
    the whole array when idx is None)."""
    segments = []
    for e in range(E):
        n, lm = SPECS[e]
        groups = []
        if e in (0, 2):
            for c in range(C):
                groups.append(dict(key=f"e{e}m", idx=c, nu=6))
            groups.append(dict(key=f"e{e}p", idx=None, nu=8))
        elif e == 1:
            for c in range(C):
                groups.append(dict(key="e1m", idx=c, nu=4))
        else:
            for cp in range(C // 2):
                groups.append(dict(key="e3m", idx=cp, nu=2))
            groups.append(dict(key="e3p", idx=None, nu=4))
        segments.append(groups)
    total_units = sum(g["nu"] for segs in segments for g in segs)
    return segments, total_units


# ------------------------------------------------------------ host packing --
def _pack_element(f, idx, e, cap):
    """Pack one tensor (f0 or f1) for element e, one core. Returns dict of
    packed arrays (float32; caller casts to bf16)."""
    n, lm = SPECS[e]
    nk = len(idx)
    x = np.zeros((cap, C, n, lm), np.float32)
    x[:nk] = f[idx][:, :, :n, :lm]
    out = {}
    if e in (0, 2):
        # main: [c, 128, 6*cap]; row = 32*j + b (m = 4q+j); col = q*cap + slot
        y = x[:, :, :, :24].reshape(cap, C, 32, 6, 4)          # slot,c,b,q,j
        out[f"e{e}m"] = np.ascontiguousarray(
            y.transpose(1, 4, 2, 3, 0).reshape(C, 128, 6 * cap))
        # packed m=24: [128, 8*cap]; row = 32*j + b (c = 4t+j); col = t*cap+slot
        z = x[:, :, :, 24].reshape(cap, 8, 4, 32)               # slot,t,j,b
        out[f"e{e}p"] = np.ascontiguousarray(
            z.transpose(2, 3, 1, 0).reshape(128, 8 * cap))
    elif e == 1:
        # [c, 128, 4*cap]; row = 32*j + b (b<24, rows 24:32 zero); m = 4q+j
        y = x.reshape(cap, C, 24, 4, 4)                          # slot,c,b,q,j
        arr = np.zeros((C, 4, 32, 4, cap), np.float32)
        arr[:, :, :24] = y.transpose(1, 4, 2, 3, 0)
        out["e1m"] = np.ascontiguousarray(arr.reshape(C, 128, 4 * cap))
    else:
        # main m<8: per c tile [128, cap]; row = 32*j + 16*s + b (m = 2j+s)
        y = x[:, :, :, :8].reshape(cap, C, 16, 4, 2)             # slot,c,b,j,s
        v = y.transpose(1, 3, 4, 2, 0).reshape(C, 128, cap)      # c,(j,s,b),slot
        # pair consecutive c's into one tile [128, 2*cap]
        out["e3m"] = np.ascontiguousarray(
            v.reshape(C // 2, 2, 128, cap).transpose(0, 2, 1, 3)
            .reshape(C // 2, 128, 2 * cap))
        # packed m=8: [128, 4*cap]; row = 32*j + 16*s + b (c = 8t + 2j + s)
        z = x[:, :, :, 8].reshape(cap, 4, 4, 2, 16)              # slot,t,j,s,b
        out["e3p"] = np.ascontiguousarray(
            z.transpose(2, 3, 4, 1, 0).reshape(128, 4 * cap))
    return out


def _build_A(ws, cts, caps):
    """lhsT tiles for every unit, in schedule order. [128, 32*U] float32."""
    segments, U = _schedule(caps)
    A = [np.einsum("cab,abm->cabm", ws[e], cts[e]).astype(np.float32)
         for e in range(E)]
    out = np.zeros((128, 32 * U), np.float32)
    u = 0
    for e in range(E):
        n, lm = SPECS[e]
        Ae = A[e]
        for g in segments[e]:
            for s in range(g["nu"]):
                tile = np.zeros((4, 32, 32), np.float32)  # j, row, col
                if g["key"].endswith("m") and e in (0, 1, 2):
                    c, q = g["idx"], s
                    for j in range(4):
                        m = 4 * q + j
                        tile[j, :n, :n] = Ae[c, :, :, m].T
                elif g["key"] == f"e{e}p" and e in (0, 2):
                    for j in range(4):
                        tile[j, :, :] = Ae[4 * s + j, :, :, 24].T
                elif g["key"] == "e3m":
                    c = 2 * g["idx"] + s
                    for j in range(4):
                        for t in range(2):
                            tile[j, 16 * t:16 * t + 16, 16 * t:16 * t + 16] = \
                                Ae[c, :, :, 2 * j + t].T
                else:  # e3p
                    for j in range(4):
                        for t in range(2):
                            c = 8 * s + 2 * j + t
                            tile[j, 16 * t:16 * t + 16, 16 * t:16 * t + 16] = \
                                Ae[c, :, :, 8].T
                out[:, 32 * u:32 * u + 32] = tile.reshape(128, 32)
                u += 1
    assert u == U
    return out


# ------------------------------------------------------------ bass builder --
def _build_nc(caps):
    import concourse.bacc as bacc
    import concourse.tile as tile
    from concourse import mybir

    segments, U = _schedule(caps)
    nc = bacc.Bacc(None, target_bir_lowering=False)
    bf = mybir.dt.bfloat16
    f32 = mybir.dt.float32

    params = {}
    shapes = {
        "e0m": (C, 128, 6 * caps[0]), "e0p": (128, 8 * caps[0]),
        "e1m": (C, 128, 4 * caps[1]),
        "e2m": (C, 128, 6 * caps[2]), "e2p": (128, 8 * caps[2]),
        "e3m": (C // 2, 128, 2 * caps[3]), "e3p": (128, 4 * caps[3]),
    }
    for t in ("f0", "f1"):
        for k, shp in shapes.items():
            name = f"{t}_{k}"
            params[name] = nc.declare_dram_parameter(name, list(shp), bf, isOutput=False)
    params["A"] = nc.declare_dram_parameter("A", [128, 32 * U], bf, isOutput=False)
    outs = []
    for e in range(E):
        outs.append(nc.declare_dram_parameter(f"out{e}", [4, caps[e]], f32, isOutput=True))

    from contextlib import ExitStack

    with tile.TileContext(nc) as tc, ExitStack() as ctx:
        pools = {}
        pools["A"] = ctx.enter_context(tc.tile_pool(name="A", bufs=1))
        pools["const"] = ctx.enter_context(tc.tile_pool(name="const", bufs=1))
        for t in ("f0", "f1"):
            for k, shp in shapes.items():
                nbuf = 4 if shp[0] in (C, C // 2) else 2
                pools[f"{t}_{k}"] = ctx.enter_context(
                    tc.tile_pool(name=f"{t}_{k}", bufs=nbuf))
        pools["g"] = ctx.enter_context(tc.tile_pool(name="g", bufs=2, space="PSUM"))
        pools["acc"] = ctx.enter_context(tc.tile_pool(name="acc", bufs=2, space="PSUM"))
        pools["h"] = ctx.enter_context(tc.tile_pool(name="h", bufs=6))
        pools["osb"] = ctx.enter_context(tc.tile_pool(name="osb", bufs=2))

        A_sb = pools["A"].tile([128, 32 * U], bf)
        nc.sync.dma_start(A_sb[:, :], params["A"][:, :])
        ones = pools["const"].tile([128, 1], bf)
        nc.any.memset(ones[:, :], 1.0)

        u = 0  # global unit counter (A column index)
        for e in range(E):
            cap = caps[e]
            nu_e = sum(g["nu"] for g in segments[e])
            assert nu_e % 4 == 0
            acc = pools["acc"].tile([128, 512], f32)
            a_ctr = 0  # acc-matmul counter within the element
            for g in segments[e]:
                key, idx, nu = g["key"], g["idx"], g["nu"]
                shp = shapes[key]
                cols = shp[-1]
                ft = {}
                for t in ("f0", "f1"):
                    tl = pools[f"{t}_{key}"].tile([128, cols], bf)
                    src = params[f"{t}_{key}"]
                    nc.sync.dma_start(tl[:, :], src[idx, :, :] if idx is not None else src[:, :])
                    ft[t] = tl
                for p in range(nu // 2):
                    gt = pools["g"].tile([128, 2, 512], f32)
                    for s in range(2):
                        col0 = (2 * p + s) * cap
                        au = u + 2 * p + s
                        for j in range(4):
                            nc.tensor.matmul(
                                gt[32 * j:32 * j + 32, s, 0:cap],
                                lhsT=A_sb[32 * j:32 * j + 32, 32 * au:32 * au + 32],
                                rhs=ft["f1"][32 * j:32 * j + 32, col0:col0 + cap],
                                start=True, stop=True,
                                tile_position=(32 * j, 32 * j),
                            )
                    ht = pools["h"].tile([128, 2, cap], bf)
                    f0v = ft["f0"][:, 2 * p * cap:(2 * p + 2) * cap].rearrange(
                        "p (s n) -> p s n", s=2)
                    nc.vector.tensor_mul(ht[:, :, :], f0v, gt[:, :, 0:cap])
                    for s in range(2):
                        grp = a_ctr % 4
                        nc.tensor.matmul(
                            acc[32 * grp:32 * grp + 1, 0:cap],
                            lhsT=ones[:, 0:1],
                            rhs=ht[:, s, :],
                            start=(a_ctr < 4), stop=(a_ctr >= nu_e - 4),
                            tile_position=(0, 32 * grp),
                        )
                        a_ctr += 1
                u += nu
            # evict the 4 accumulator rows
            ot = pools["osb"].tile([128, 512], f32)
            for grp in range(4):
                nc.vector.tensor_copy(ot[32 * grp:32 * grp + 1, 0:cap],
                                      acc[32 * grp:32 * grp + 1, 0:cap])
                nc.sync.dma_start(outs[e][grp:grp + 1, :],
                                  ot[32 * grp:32 * grp + 1, 0:cap])
        assert u == U
    nc.compile()
    return nc


# ------------------------------------------------------------------ kernel --
def _install_ntff_shim():
    """antenv.axon_hooks is missing from this image; recreate it so
    run_bass_kernel_spmd(trace=True) can capture NTFF profiles under axon.
    Only used when KERNEL_TRACE=1 (local timing runs)."""
    import sys, types
    if "antenv.axon_hooks" in sys.modules:
        return
    import antenv
    mod = types.ModuleType("antenv.axon_hooks")
    mod._hook = None
    mod.set_axon_ntff_profile_hook = lambda h: setattr(mod, "_hook", h)
    mod.get_axon_ntff_profile_hook = lambda: mod._hook
    sys.modules["antenv.axon_hooks"] = mod
    antenv.axon_hooks = mod
    from trn_agent_boot.trn_boot import _ntff_profile_via_ctypes
    hook = _ntff_profile_via_ctypes("/opt/axon/libaxon_pjrt.so")
    if hook is not None:
        mod.set_axon_ntff_profile_hook(hook)


def kernel(**inputs):
    f0 = np.asarray(inputs["f0"], np.float32)
    f1 = np.asarray(inputs["f1"], np.float32)
    eids = np.asarray(inputs["element_ids"])
    ws = [np.asarray(inputs[f"w{e}"], np.float32) for e in range(E)]
    cts = [np.asarray(inputs[f"ct{e}"], np.float32) for e in range(E)]

    order = np.argsort(eids, kind="stable")
    idx_e = [order[eids[order] == e] for e in range(E)]
    counts = [len(x) for x in idx_e]
    caps = [max(2, -(-c // NCORES)) for c in counts]
    caps = [cap + (cap % 2) for cap in caps]  # even
    assert all(cap <= 512 for cap in caps)

    assign = []  # assign[e][k] = atom indices for core k
    for e in range(E):
        k_e = counts[e]
        base, rem = divmod(k_e, NCORES)
        sizes = [base + (1 if k < rem else 0) for k in range(NCORES)]
        offs = np.cumsum([0] + sizes)
        assign.append([idx_e[e][offs[k]:offs[k + 1]] for k in range(NCORES)])

    A_np = _build_A(ws, cts, caps).astype(BF16)
    in_maps = []
    for k in range(NCORES):
        m = {"A": A_np}
        for e in range(E):
            for t, f in (("f0", f0), ("f1", f1)):
                packed = _pack_element(f, assign[e][k], e, caps[e])
                for key, arr in packed.items():
                    m[f"{t}_{key}"] = arr.astype(BF16)
        in_maps.append(m)

    from concourse.bass_utils import run_bass_kernel_spmd

    nc = _build_nc(caps)
    trace = bool(int(os.environ.get("KERNEL_TRACE", "0")))
    if trace:
        try:
            _install_ntff_shim()
        except Exception:
            pass
    res = run_bass_kernel_spmd(nc, in_maps, core_ids=list(range(NCORES)), trace=trace)
    if trace and res.exec_time_ns is not None:
        print(f"HW exec time: {res.exec_time_ns} ns")

    out = np.zeros((N_ATOMS,), np.float32)
    for k in range(NCORES):
        for e in range(E):
            vals = res.results[k][f"out{e}"].sum(axis=0)
            nk = len(assign[e][k])
            out[assign[e][k]] = vals[:nk]
    return out[:, None].astype(np.float32)


# revision 12
# speedup vs baseline: 1.1597x; 1.1597x over previous
"""ACE layer (moe_routing) Trainium2 kernel — 8 NeuronCores.

out[i] = sum_{c,a,b,m} w_e[c,a,b] * ct_e[a,b,m] * f0[i,c,a,m] * f1[i,c,b,m],  e = element_ids[i]

Strategy (v3, "unit-sharded"):
  The contraction decomposes into 568 independent "units": each unit is a
  [128 x 128] block-diagonal weight tile (4 blocks of 32 covering 4 m-values /
  packed (c,m) combos of one element) applied to all atoms of that element.
    el0/el2 (n=32,lm=25): per c: 6 units (m 4q..4q+3) + 8 c-packed m=24 units
    el1     (n=24,lm=16): per c: 4 units (rows/cols 24:32 zero)
    el3     (n=16,lm=9) : per c: 1 unit (blocks = 2 m's stacked) + 4 c-packed
                          m=8 units + 4 zero dummy units
  Each core owns 71 units (25 el0 + 16 el1 + 25 el2 + 5 el3) and streams ALL
  atoms of the unit's element through them: one K=128,M=128,N<=512 matmul per
  512-atom chunk (G = A^T f1, PSUM), one DVE multiply h = f0*G (bf16), one
  ones-matmul (K=128,M=1) accumulating sum_p h[p,i] into a per-(element,chunk)
  PSUM row (column-group rotated by unit). Host sums partial outputs across
  cores/groups and scatters back to atom order. All f0/f1 data moves as bf16.
"""

import math
import os

import numpy as np
import ml_dtypes

BF16 = ml_dtypes.bfloat16
SPECS = [(32, 25), (24, 16), (32, 25), (16, 9)]
N_ATOMS, C, NMAX, LMMAX, E = 8192, 32, 32, 25, 4
NCORES = 8
NPAD = 2080                      # padded atom count per element
CHUNKS = [512, 512, 512, 512, 32]  # per-unit matmul chunks (PSUM bank = 512 f32)
UNITS_PER_ELEM = [200, 128, 200, 40]   # el3 includes 4 zero dummy units
SEG_LEN = [u // NCORES for u in UNITS_PER_ELEM]  # per-core: [25, 16, 25, 5]


def _unit_descs():
    """Global unit list in order; desc = (e, kind, c_or_t, q)."""
    units = []
    for e in range(E):
        if e in (0, 2):
            for c in range(C):
                for q in range(6):
                    units.append((e, "m", c, q))
            for t in range(8):
                units.append((e, "p", t, 0))
        elif e == 1:
            for c in range(C):
                for q in range(4):
                    units.append((e, "m", c, q))
        else:
            for c in range(C):
                units.append((e, "m", c, 0))
            for t in range(4):
                units.append((e, "p", t, 0))
            for t in range(4):
                units.append((e, "d", t, 0))  # dummy
    assert len(units) == sum(UNITS_PER_ELEM)
    return units


def _unit_blocks(desc, ws_ct):
    """[4, 32, 32] lhsT blocks (row=K/b side, col=M/a side) for one unit."""
    e, kind, idx, q = desc
    n, lm = SPECS[e]
    Ae = ws_ct[e]
    tile = np.zeros((4, 32, 32), np.float32)
    if kind == "d":
        return tile
    if kind == "m" and e in (0, 1, 2):
        for j in range(4):
            tile[j, :n, :n] = Ae[idx, :, :, 4 * q + j].T
    elif kind == "p" and e in (0, 2):
        for j in range(4):
            tile[j, :, :] = Ae[4 * idx + j, :, :, 24].T
    elif kind == "m":  # e3 main
        for j in range(4):
            for s in range(2):
                tile[j, 16 * s:16 * s + 16, 16 * s:16 * s + 16] = \
                    Ae[idx, :, :, 2 * j + s].T
    else:  # e3 packed m=8
        for j in range(4):
            for s in range(2):
                c = 8 * idx + 2 * j + s
                tile[j, 16 * s:16 * s + 16, 16 * s:16 * s + 16] = Ae[c, :, :, 8].T
    return tile


def _pack_f_element(f, idx, e):
    """Pack one tensor for element e into per-unit-kind arrays.

    Returns dict: 'm' -> [n_main_units, 128(or 96 for el1: 4*24), NPAD],
                  'p' -> [n_packed, 128, NPAD] (el0/el2/el3 only)."""
    n, lm = SPECS[e]
    k = len(idx)
    x = np.zeros((NPAD, C, n, lm), np.float32)
    x[:k] = f[idx][:, :, :n, :lm]
    out = {}
    if e in (0, 2):
        y = x[:, :, :, :24].reshape(NPAD, C, 32, 6, 4)       # slot,c,b,q,j
        # main units ordered (c, q): [c, q, j, b, slot] -> [c*6+q, 128, NPAD]
        out["m"] = np.ascontiguousarray(
            y.transpose(1, 3, 4, 2, 0).reshape(C * 6, 128, NPAD))
        z = x[:, :, :, 24].reshape(NPAD, 8, 4, 32)           # slot,t,j,b
        out["p"] = np.ascontiguousarray(
            z.transpose(1, 2, 3, 0).reshape(8, 128, NPAD))
    elif e == 1:
        # rows 24:32 of each block are NOT stored (DMA'd per block, 24 rows)
        y = x.reshape(NPAD, C, 24, 4, 4)                     # slot,c,b,q,j
        out["m"] = np.ascontiguousarray(
            y.transpose(1, 3, 4, 2, 0).reshape(C * 4, 4, 24, NPAD))
    else:
        y = x[:, :, :, :8].reshape(NPAD, C, 16, 4, 2)        # slot,c,b,j,s
        out["m"] = np.ascontiguousarray(
            y.transpose(1, 3, 4, 2, 0).reshape(C, 128, NPAD))
        z = x[:, :, :, 8].reshape(NPAD, 4, 4, 2, 16)         # slot,t,j,s,b
        out["p"] = np.ascontiguousarray(
            z.transpose(1, 2, 3, 4, 0).reshape(4, 128, NPAD))
    return out


# ------------------------------------------------------------ bass builder --
def _build_nc():
    import concourse.bacc as bacc
    import concourse.tile as tile
    from concourse import mybir
    from contextlib import ExitStack

    nc = bacc.Bacc(None, target_bir_lowering=False)
    bf = mybir.dt.bfloat16
    f32 = mybir.dt.float32
    UPC = sum(SEG_LEN)  # units per core = 71

    params = {}
    # f params per element (per-core unit slices)
    fshape = {0: [SEG_LEN[0], 128, NPAD], 1: [SEG_LEN[1], 4, 24, NPAD],
              2: [SEG_LEN[2], 128, NPAD], 3: [SEG_LEN[3], 128, NPAD]}
    for t in ("f0", "f1"):
        for e in range(E):
            name = f"{t}_e{e}"
            params[name] = nc.declare_dram_parameter(name, fshape[e], bf, isOutput=False)
    params["A"] = nc.declare_dram_parameter("A", [128, 128 * UPC], bf, isOutput=False)
    outs = [nc.declare_dram_parameter(f"out{e}", [len(CHUNKS), 4, 512], f32,
                                      isOutput=True) for e in range(E)]

    with tile.TileContext(nc) as tc, ExitStack() as ctx:
        pA = ctx.enter_context(tc.tile_pool(name="A", bufs=1))
        pconst = ctx.enter_context(tc.tile_pool(name="const", bufs=1))
        pf0 = ctx.enter_context(tc.tile_pool(name="f0", bufs=3))
        pf1 = ctx.enter_context(tc.tile_pool(name="f1", bufs=3))
        pf0e1 = ctx.enter_context(tc.tile_pool(name="f0e1", bufs=3))
        pf1e1 = ctx.enter_context(tc.tile_pool(name="f1e1", bufs=3))
        pg = ctx.enter_context(tc.tile_pool(name="g", bufs=3, space="PSUM"))
        pacc = ctx.enter_context(tc.tile_pool(name="acc", bufs=1, space="PSUM"))
        ph = ctx.enter_context(tc.tile_pool(name="h", bufs=5))
        pst = ctx.enter_context(tc.tile_pool(name="stage", bufs=2))

        A_sb = pA.tile([128, 128 * UPC], bf)
        nc.sync.dma_start(A_sb[:, :], params["A"][:, :])
        ones = pconst.tile([128, 1], bf)
        nc.any.memset(ones[:, :], 1.0)

        # pre-zero the el1 pools' pad rows (rows 24:32 of each 32-block stay
        # zero forever; DMA only writes rows :24)
        for pool, tag in ((pf0e1, "te0"), (pf1e1, "te1")):
            for _ in range(3):
                t = pool.tile([128, NPAD], bf, tag=tag)
                nc.any.memset(t[:, :], 0.0)

        ucol = 0  # column offset into A_sb, advances per unit
        for e in range(E):
            seg = SEG_LEN[e]
            accs = [pacc.tile([128, 512], f32, name=f"acc{ci}", tag=f"acc{ci}")
                    for ci in range(len(CHUNKS))]
            for u in range(seg):
                if e == 1:
                    f0t = pf0e1.tile([128, NPAD], bf, tag="te0")
                    f1t = pf1e1.tile([128, NPAD], bf, tag="te1")
                    for j in range(4):
                        nc.sync.dma_start(f0t[32 * j:32 * j + 24, :],
                                          params["f0_e1"][u, j, :, :])
                        nc.sync.dma_start(f1t[32 * j:32 * j + 24, :],
                                          params["f1_e1"][u, j, :, :])
                else:
                    f0t = pf0.tile([128, NPAD], bf)
                    f1t = pf1.tile([128, NPAD], bf)
                    nc.sync.dma_start(f0t[:, :], params[f"f0_e{e}"][u, :, :])
                    nc.sync.dma_start(f1t[:, :], params[f"f1_e{e}"][u, :, :])
                grp = u % 4
                off = 0
                for ci, cn in enumerate(CHUNKS):
                    g = pg.tile([128, 512], f32)
                    nc.tensor.matmul(
                        g[:, 0:cn],
                        lhsT=A_sb[:, 128 * ucol:128 * (ucol + 1)],
                        rhs=f1t[:, off:off + cn],
                        start=True, stop=True)
                    h = ph.tile([128, 512], bf)
                    nc.vector.tensor_mul(h[:, 0:cn], f0t[:, off:off + cn],
                                         g[:, 0:cn])
                    nc.tensor.matmul(
                        accs[ci][32 * grp:32 * grp + 1, 0:cn],
                        lhsT=ones[:, 0:1], rhs=h[:, 0:cn],
                        start=(u < 4), stop=(u >= seg - 4),
                        tile_position=(0, 32 * grp))
                    off += cn
                ucol += 1
            # evict: 4 accumulator rows per chunk -> stage -> DRAM
            for ci, cn in enumerate(CHUNKS):
                st = pst.tile([128, 512], f32)
                for grp in range(4):
                    nc.vector.tensor_copy(st[32 * grp:32 * grp + 1, 0:cn],
                                          accs[ci][32 * grp:32 * grp + 1, 0:cn])
                nc.sync.dma_start(outs[e][ci, :, 0:cn], st[0:97:32, 0:cn])
        assert ucol == UPC
    nc.compile()
    return nc


# ------------------------------------------------------------------ kernel --
def _install_ntff_shim():
    """antenv.axon_hooks is missing from this image; recreate it so
    run_bass_kernel_spmd(trace=True) can capture NTFF profiles under axon.
    Only used when KERNEL_TRACE=1 (local timing runs)."""
    import sys, types
    if "antenv.axon_hooks" in sys.modules:
        return
    import antenv
    mod = types.ModuleType("antenv.axon_hooks")
    mod._hook = None
    mod.set_axon_ntff_profile_hook = lambda h: setattr(mod, "_hook", h)
    mod.get_axon_ntff_profile_hook = lambda: mod._hook
    sys.modules["antenv.axon_hooks"] = mod
    antenv.axon_hooks = mod
    from trn_agent_boot.trn_boot import _ntff_profile_via_ctypes
    hook = _ntff_profile_via_ctypes("/opt/axon/libaxon_pjrt.so")
    if hook is not None:
        mod.set_axon_ntff_profile_hook(hook)


def kernel(**inputs):
    f0 = np.asarray(inputs["f0"], np.float32)
    f1 = np.asarray(inputs["f1"], np.float32)
    eids = np.asarray(inputs["element_ids"])
    ws = [np.asarray(inputs[f"w{e}"], np.float32) for e in range(E)]
    cts = [np.asarray(inputs[f"ct{e}"], np.float32) for e in range(E)]

    order = np.argsort(eids, kind="stable")
    idx_e = [order[eids[order] == e] for e in range(E)]
    counts = [len(x) for x in idx_e]
    assert all(c <= NPAD for c in counts)

    ws_ct = [np.einsum("cab,abm->cabm", ws[e], cts[e]).astype(np.float32)
             for e in range(E)]
    units = _unit_descs()

    # global A: [n_units, 128, 128] block-diagonal, then per-core slices
    A_glob = np.zeros((len(units), 128, 128), np.float32)
    for ui, desc in enumerate(units):
        blocks = _unit_blocks(desc, ws_ct)
        for j in range(4):
            A_glob[ui, 32 * j:32 * j + 32, 32 * j:32 * j + 32] = blocks[j]
    A_glob = A_glob.astype(BF16)

    # global packed f arrays per element, concatenated in unit order
    packed = {}
    for t, f in (("f0", f0), ("f1", f1)):
        for e in range(E):
            pk = _pack_f_element(f, idx_e[e], e)
            parts = [pk["m"].astype(BF16)]
            if "p" in pk:
                parts.append(pk["p"].astype(BF16))
            n_have = sum(p.shape[0] for p in parts)
            if n_have < UNITS_PER_ELEM[e]:  # el3 dummy units
                zshape = (UNITS_PER_ELEM[e] - n_have,) + parts[0].shape[1:]
                parts.append(np.zeros(zshape, BF16))
            full = np.concatenate(parts, axis=0) if len(parts) > 1 else parts[0]
            assert full.shape[0] == UNITS_PER_ELEM[e]
            packed[(t, e)] = full

    # per-core in_maps: unit slices. Unit order: element-major, so each core's
    # slice of element e's units is [k*SEG_LEN[e], (k+1)*SEG_LEN[e]).
    UPC = sum(SEG_LEN)
    in_maps = []
    for k in range(NCORES):
        m = {}
        # A slice: this core's units, in program order
        rows = []
        base = 0
        for e in range(E):
            n_e = UNITS_PER_ELEM[e]
            rows.append(A_glob[base + k * SEG_LEN[e]: base + (k + 1) * SEG_LEN[e]])
            base += n_e
        A_core = np.concatenate(rows, axis=0)              # [71, 128, 128]
        m["A"] = np.ascontiguousarray(
            A_core.transpose(1, 0, 2).reshape(128, 128 * UPC))
        for t in ("f0", "f1"):
            for e in range(E):
                arr = packed[(t, e)][k * SEG_LEN[e]:(k + 1) * SEG_LEN[e]]
                assert arr.shape[0] == SEG_LEN[e], (t, e, k, arr.shape)
                m[f"{t}_e{e}"] = arr
        in_maps.append(m)

    from concourse.bass_utils import run_bass_kernel_spmd

    nc = _build_nc()
    trace = bool(int(os.environ.get("KERNEL_TRACE", "0")))
    if trace:
        try:
            _install_ntff_shim()
        except Exception:
            pass
    res = run_bass_kernel_spmd(nc, in_maps, core_ids=list(range(NCORES)), trace=trace)
    if trace and res.exec_time_ns is not None:
        print(f"HW exec time: {res.exec_time_ns} ns")

    out = np.zeros((N_ATOMS,), np.float32)
    for e in range(E):
        tot = np.zeros((len(CHUNKS), 512), np.float64)
        for k in range(NCORES):
            tot += res.results[k][f"out{e}"].astype(np.float64).sum(axis=1)
        vals = tot.reshape(-1)[:counts[e]].astype(np.float32)
        out[idx_e[e]] = vals
    return out[:, None].astype(np.float32)
